# revision 7
# baseline (speedup 1.0000x reference)
"""Causal self-attention (B=2, L=2048, HID=2048, H=16, D=128) on 8 trn2 cores.

Sharding: core c -> (batch b = c//4, head-group g = c%4 of 4 heads).
Each core computes q/k/v projections for its 512 features from its batch,
RoPE, causal attention for its 4 heads, and a partial output projection
against its Wo column slice. Host sums the 4 partials per batch.

All matmuls run in float32r (RNE-to-11-mantissa-bit inputs, fp32 accumulate).
"""
import numpy as np

import concourse.mybir as mybir
import concourse.tile as tile
from concourse import bacc
from concourse.bass_utils import run_bass_kernel_spmd

B, L, HID, H = 2, 2048, 2048, 16
D = 128               # head dim
NCORES = 8
GH = 4                # heads per core
E = GH * D            # 512 per-core qkv features
NT = HID // 128       # 16 contraction tiles
NI = L // 512         # 4 i-chunks of 512
SCALE = 1.0 / float(np.sqrt(D))

F32 = mybir.dt.float32
F32R = mybir.dt.float32r
MULT = mybir.AluOpType.mult
ADD = mybir.AluOpType.add
IS_GE = mybir.AluOpType.is_ge


def _emit(nc, tc, ctx, io):
    xT, wqT, wkT, wvT, woT, cosT, sinT, rotT, out = (
        io["xT"], io["wqT"], io["wkT"], io["wvT"], io["woT"],
        io["cosT"], io["sinT"], io["rotT"], io["out"],
    )
    xTr = xT.rearrange("(t p) i -> p t i", p=128)       # [128, 16, 2048]
    wqTr = wqT.rearrange("(t p) e -> p t e", p=128)     # [128, 16, 512]
    wkTr = wkT.rearrange("(t p) e -> p t e", p=128)
    wvTr = wvT.rearrange("(t p) e -> p t e", p=128)
    woTr = woT.rearrange("(s p) f -> p s f", p=128)     # [128, 4, 2048]

    # persistent across (nearly) the whole kernel
    persist = ctx.enter_context(tc.tile_pool(name="persist", bufs=1))
    vpool = ctx.enter_context(tc.tile_pool(name="vpool", bufs=1))

    ones_f = persist.tile([128, 1], F32, tag="ones_f")
    nc.gpsimd.memset(ones_f[:], 1.0)
    ones = persist.tile([128, 1], F32R, tag="ones")
    nc.vector.tensor_copy(ones[:], ones_f[:])
    rot = persist.tile([128, 128], F32R, tag="rot")
    nc.sync.dma_start(rot[:], rotT)
    qr = [persist.tile([128, L], F32R, tag=f"qr{h}", name=f"qr{h}") for h in range(GH)]
    kr = [persist.tile([128, L], F32R, tag=f"kr{h}", name=f"kr{h}") for h in range(GH)]
    v_sb = [vpool.tile([128, E], F32R, tag=f"v{jt}", name=f"v{jt}") for jt in range(NT)]

    with tc.tile_pool(name="xst", bufs=4) as xpool, \
         tc.tile_pool(name="work", bufs=2) as work, \
         tc.tile_pool(name="psAB", bufs=4, space="PSUM") as psAB:

        def load_slab(ic, mt):
            """xT[mt-tile, ic*512:+512] as [128, 512]."""
            xsl = xpool.tile([128, 512], F32R, tag="xsl", name="xsl")
            nc.sync.dma_start(xsl[:], xTr[:, mt, ic * 512 : (ic + 1) * 512])
            return xsl

        # ---------------- phase A: V projection -> V[j, e] tiles -------------
        with tc.tile_pool(name="wvp", bufs=1) as wvpool:
            wv_sb = wvpool.tile([128, NT, 512], F32R, tag="wv")
            for c in range(4):
                nc.sync.dma_start(wv_sb[:, 4 * c : 4 * c + 4, :], wvTr[:, 4 * c : 4 * c + 4, :])
            for icj in range(NI):
                vps = [psAB.tile([128, 512], F32, tag="mm", name=f"vp{jt}") for jt in range(4)]
                for mt in range(NT):
                    xsl = load_slab(icj, mt)
                    for jt in range(4):
                        nc.tensor.matmul(
                            vps[jt][:],
                            xsl[:, jt * 128 : (jt + 1) * 128],
                            wv_sb[:, mt, :],
                            start=(mt == 0),
                            stop=(mt == NT - 1),
                        )
                for jt in range(4):
                    nc.scalar.copy(v_sb[4 * icj + jt][:], vps[jt][:])

        # ------------- phase B: Q then K projection + RoPE -------------------
        with tc.tile_pool(name="csp", bufs=1) as cspool:
            cos_sb = cspool.tile([128, L], F32, tag="cos")
            sin_sb = cspool.tile([128, L], F32, tag="sin")
            nc.sync.dma_start(cos_sb[:], cosT)
            nc.sync.dma_start(sin_sb[:], sinT)

            for wdram, dst, wtag in ((wqTr, qr, "wq"), (wkTr, kr, "wk")):
                with tc.tile_pool(name=f"{wtag}p", bufs=1) as wpool:
                    w_sb = wpool.tile([128, NT, 512], F32R, tag=wtag, name=wtag)
                    for c in range(4):
                        nc.sync.dma_start(
                            w_sb[:, 4 * c : 4 * c + 4, :], wdram[:, 4 * c : 4 * c + 4, :]
                        )
                    for ic in range(NI):
                        isl = slice(ic * 512, (ic + 1) * 512)
                        pps = [
                            psAB.tile([128, 512], F32, tag="mm", name=f"pp{dt}")
                            for dt in range(GH)
                        ]
                        for mt in range(NT):
                            xsl = load_slab(ic, mt)
                            for dt in range(GH):
                                nc.tensor.matmul(
                                    pps[dt][:],
                                    w_sb[:, mt, dt * 128 : (dt + 1) * 128],
                                    xsl[:],
                                    start=(mt == 0),
                                    stop=(mt == NT - 1),
                                )
                        for dt in range(GH):
                            pre = work.tile([128, 512], F32R, tag="pre")
                            nc.vector.tensor_copy(pre[:], pps[dt][:])
                            rp = psAB.tile([128, 512], F32, tag="rot", bufs=2)
                            nc.tensor.matmul(rp[:], rot[:], pre[:], start=True, stop=True)
                            t1 = work.tile([128, 512], F32, tag="t1")
                            nc.vector.tensor_tensor(t1[:], pre[:], cos_sb[:, isl], MULT)
                            t2 = work.tile([128, 512], F32, tag="t2")
                            nc.vector.tensor_tensor(t2[:], rp[:], sin_sb[:, isl], MULT)
                            nc.vector.tensor_tensor(dst[dt][:, isl], t1[:], t2[:], ADD)

    # ------------- phase C: causal attention per head --------------------
    with tc.tile_pool(name="otp", bufs=1) as otpool:
        ot = [otpool.tile([128, L], F32R, tag=f"ot{h}", name=f"ot{h}") for h in range(GH)]
        with tc.tile_pool(name="Ep", bufs=1) as epool, \
             tc.tile_pool(name="dp", bufs=2) as dpool, \
             tc.tile_pool(name="psC", bufs=1, space="PSUM") as psC:
            for h in range(GH):
                for I in range(NI):
                    nj = (I + 1) * 4
                    i0 = I * 512
                    isl = slice(i0, i0 + 512)
                    e_tiles = []
                    for jt in range(nj):
                        st = psC.tile([128, 512], F32, tag="st", bufs=3)
                        nc.tensor.matmul(
                            st[:],
                            kr[h][:, jt * 128 : (jt + 1) * 128],
                            qr[h][:, isl],
                            start=True,
                            stop=True,
                        )
                        et = epool.tile([128, 512], F32R, tag=f"E{jt}", name=f"et{jt}")
                        nc.scalar.activation(
                            et[:], st[:], mybir.ActivationFunctionType.Exp, scale=SCALE
                        )
                        if jt >= I * 4:
                            # keep where i >= j: (i0 + c) - (jt*128 + p) >= 0
                            nc.gpsimd.affine_select(
                                out=et[:],
                                in_=et[:],
                                compare_op=IS_GE,
                                fill=0.0,
                                base=i0 - jt * 128,
                                pattern=[[1, 512]],
                                channel_multiplier=-1,
                            )
                        e_tiles.append(et)
                    ov = psC.tile([128, 512], F32, tag="ov", bufs=2)
                    for jt in range(nj):
                        nc.tensor.matmul(
                            ov[:],
                            v_sb[jt][:, h * 128 : (h + 1) * 128],
                            e_tiles[jt][:],
                            start=(jt == 0),
                            stop=(jt == nj - 1),
                        )
                    dn = psC.tile([1, 512], F32, tag="dn", bufs=2)
                    for jt in range(nj):
                        nc.tensor.matmul(
                            dn[:], ones[:], e_tiles[jt][:],
                            start=(jt == 0), stop=(jt == nj - 1),
                        )
                    den_sb = dpool.tile([1, 512], F32, tag="den")
                    nc.vector.tensor_copy(den_sb[:], dn[:])
                    rb = dpool.tile([128, 512], F32, tag="rb")
                    nc.gpsimd.partition_broadcast(rb[:], den_sb[:], channels=128)
                    rbi = dpool.tile([128, 512], F32, tag="rbi")
                    nc.vector.reciprocal(rbi[:], rb[:])
                    nc.vector.tensor_tensor(ot[h][:, isl], ov[:], rbi[:], MULT)
        # ------------- phase D: partial output projection --------------------
        with tc.tile_pool(name="wop", bufs=1) as wopool, \
             tc.tile_pool(name="obp", bufs=3) as obpool, \
             tc.tile_pool(name="psD", bufs=3, space="PSUM") as psD:
            wo_sb = wopool.tile([128, GH, L], F32R, tag="wo")
            for s_ in range(GH):
                nc.sync.dma_start(wo_sb[:, s_, :], woTr[:, s_, :])
            for it in range(NT):
                for fc in range(NI):
                    op = psD.tile([128, 512], F32, tag="wo")
                    for h in range(GH):
                        nc.tensor.matmul(
                            op[:],
                            ot[h][:, it * 128 : (it + 1) * 128],
                            wo_sb[:, h, fc * 512 : (fc + 1) * 512],
                            start=(h == 0),
                            stop=(h == GH - 1),
                        )
                    ob = obpool.tile([128, 512], F32, tag="ob")
                    if (it + fc) % 2 == 0:
                        nc.vector.tensor_copy(ob[:], op[:])
                    else:
                        nc.scalar.copy(ob[:], op[:])
                    nc.sync.dma_start(
                        out[it * 128 : (it + 1) * 128, fc * 512 : (fc + 1) * 512], ob[:]
                    )


def build():
    import contextlib

    nc = bacc.Bacc("TRN2", target_bir_lowering=False, debug=False, num_devices=NCORES)
    io = {
        "xT": nc.dram_tensor("xT", [HID, L], F32R, kind="ExternalInput").ap(),
        "wqT": nc.dram_tensor("wqT", [HID, E], F32R, kind="ExternalInput").ap(),
        "wkT": nc.dram_tensor("wkT", [HID, E], F32R, kind="ExternalInput").ap(),
        "wvT": nc.dram_tensor("wvT", [HID, E], F32R, kind="ExternalInput").ap(),
        "woT": nc.dram_tensor("woT", [E, HID], F32R, kind="ExternalInput").ap(),
        "cosT": nc.dram_tensor("cosT", [D, L], F32, kind="ExternalInput").ap(),
        "sinT": nc.dram_tensor("sinT", [D, L], F32, kind="ExternalInput").ap(),
        "rotT": nc.dram_tensor("rotT", [D, D], F32R, kind="ExternalInput").ap(),
        "out": nc.dram_tensor("out", [L, HID], F32, kind="ExternalOutput").ap(),
    }
    with tile.TileContext(nc) as tc:
        with contextlib.ExitStack() as ctx:
            _emit(nc, tc, ctx, io)
    nc.compile()
    return nc


_NC_CACHE = []


def _rot_matrix():
    # lhsT for the rotate_half matmul: rot(q) = P @ q, lhsT = P^T.
    rotT = np.zeros((D, D), dtype=np.float32)
    for d in range(D // 2):
        rotT[d, d + 64] = 1.0
        rotT[d + 64, d] = -1.0
    return rotT


def make_in_maps(hidden_states, cos, sin, Wq, Wk, Wv, Wo):
    f = np.float32
    cosT = np.ascontiguousarray(cos.T.astype(f))
    sinT = np.ascontiguousarray(sin.T.astype(f))
    rotT = _rot_matrix()
    xTs = [np.ascontiguousarray(hidden_states[b].T.astype(f)) for b in range(B)]
    in_maps = []
    for c in range(NCORES):
        b, g = divmod(c, 4)
        sl = slice(g * E, (g + 1) * E)
        in_maps.append({
            "xT": xTs[b],
            "wqT": np.ascontiguousarray(Wq[sl].T.astype(f)),
            "wkT": np.ascontiguousarray(Wk[sl].T.astype(f)),
            "wvT": np.ascontiguousarray(Wv[sl].T.astype(f)),
            "woT": np.ascontiguousarray(Wo[:, sl].T.astype(f)),
            "cosT": cosT,
            "sinT": sinT,
            "rotT": rotT,
        })
    return in_maps


def kernel(hidden_states, cos, sin, Wq, Wk, Wv, Wo):
    hidden_states, cos, sin, Wq, Wk, Wv, Wo = (
        np.asarray(a) for a in (hidden_states, cos, sin, Wq, Wk, Wv, Wo)
    )
    if not _NC_CACHE:
        _NC_CACHE.append(build())
    nc = _NC_CACHE[0]
    in_maps = make_in_maps(hidden_states, cos, sin, Wq, Wk, Wv, Wo)
    r = run_bass_kernel_spmd(nc, in_maps, list(range(NCORES)))
    out = np.empty((B, L, HID), np.float32)
    for b in range(B):
        acc = r.results[4 * b]["out"].astype(np.float32).copy()
        for g in range(1, 4):
            acc += r.results[4 * b + g]["out"]
        out[b] = acc
    return out


# revision 9
# speedup vs baseline: 1.2459x; 1.2459x over previous
"""Causal self-attention (B=2, L=2048, HID=2048, H=16, D=128) on 8 trn2 cores.

Sharding: core c -> (batch b = c//4, head-group g = c%4 of 4 heads).
Each core computes q/k/v projections for its 512 features from its batch,
RoPE, causal attention for its 4 heads, and a partial output projection
against its Wo column slice. Host sums the 4 partials per batch.

All matmuls run in float32r (RNE-to-11-mantissa-bit inputs, fp32 accumulate).
"""
import numpy as np

import concourse.mybir as mybir
import concourse.tile as tile
from concourse import bacc
from concourse.bass_utils import run_bass_kernel_spmd

B, L, HID, H = 2, 2048, 2048, 16
D = 128               # head dim
NCORES = 8
GH = 4                # heads per core
E = GH * D            # 512 per-core qkv features
NT = HID // 128       # 16 contraction tiles
NI = L // 512         # 4 i-chunks of 512
SCALE = 1.0 / float(np.sqrt(D))

F32 = mybir.dt.float32
F32R = mybir.dt.float32r
MULT = mybir.AluOpType.mult
ADD = mybir.AluOpType.add
IS_GE = mybir.AluOpType.is_ge
DT = mybir.dt.float16       # on-chip matmul dtype
NP_DT = np.float16
EXP_BIAS = -4.0             # exp(s*scale - 4): fp16 overflow headroom, cancels in softmax


def _emit(nc, tc, ctx, io):
    xT, wqT, wkT, wvT, woT, cosT, sinT, rotT, out = (
        io["xT"], io["wqT"], io["wkT"], io["wvT"], io["woT"],
        io["cosT"], io["sinT"], io["rotT"], io["out"],
    )
    xTr = xT.rearrange("(t p) i -> p t i", p=128)       # [128, 16, 2048]
    wqTr = wqT.rearrange("(t p) e -> p t e", p=128)     # [128, 16, 512]
    wkTr = wkT.rearrange("(t p) e -> p t e", p=128)
    wvTr = wvT.rearrange("(t p) e -> p t e", p=128)
    woTr = woT.rearrange("(s p) f -> p s f", p=128)     # [128, 4, 2048]

    pool = ctx.enter_context(tc.tile_pool(name="main", bufs=1))
    xpool = ctx.enter_context(tc.tile_pool(name="xsl", bufs=4))
    work = ctx.enter_context(tc.tile_pool(name="work", bufs=2))
    obpool = ctx.enter_context(tc.tile_pool(name="ob", bufs=3))
    dpool = ctx.enter_context(tc.tile_pool(name="dp", bufs=2))
    # single PSUM pool, exactly 8 banks: mm(4) + acc(2) + dn(2)
    ps = ctx.enter_context(tc.tile_pool(name="ps", bufs=4, space="PSUM"))

    def load_slab(ic, mt):
        xsl = xpool.tile([128, 512], DT, tag="xsl", name="xsl")
        nc.sync.dma_start(xsl[:], xTr[:, mt, ic * 512 : (ic + 1) * 512])
        return xsl

    # ---------------- phase A: V projection -> V[j, e] tiles -------------
    wv_sb = pool.tile([128, NT, 512], DT, tag="wv")
    for c in range(4):
        nc.sync.dma_start(wv_sb[:, 4 * c : 4 * c + 4, :], wvTr[:, 4 * c : 4 * c + 4, :])
    v_sb = [pool.tile([128, E], DT, tag=f"v{jt}", name=f"v{jt}") for jt in range(NT)]

    ones_f = pool.tile([128, 1], F32, tag="ones_f")
    nc.gpsimd.memset(ones_f[:], 1.0)
    ebias = pool.tile([128, 1], F32, tag="ebias")
    nc.gpsimd.memset(ebias[:], EXP_BIAS)
    ones = pool.tile([128, 1], DT, tag="ones")
    nc.vector.tensor_copy(ones[:], ones_f[:])
    rot = pool.tile([128, 128], DT, tag="rot")
    nc.sync.dma_start(rot[:], rotT)

    for icj in range(NI):
        vps = [ps.tile([128, 512], F32, tag="mm", name=f"vp{jt}") for jt in range(4)]
        for mt in range(NT):
            xsl = load_slab(icj, mt)
            for jt in range(4):
                nc.tensor.matmul(
                    vps[jt][:],
                    xsl[:, jt * 128 : (jt + 1) * 128],
                    wv_sb[:, mt, :],
                    start=(mt == 0),
                    stop=(mt == NT - 1),
                )
        for jt in range(4):
            nc.scalar.copy(v_sb[4 * icj + jt][:], vps[jt][:])
        if icj == 0:
            # prefetch q/k weights and rope tables behind phase A's compute
            cos_sb = pool.tile([128, L], F32, tag="cos")
            sin_sb = pool.tile([128, L], F32, tag="sin")
            nc.sync.dma_start(cos_sb[:], cosT)
            nc.sync.dma_start(sin_sb[:], sinT)
            wq_sb = pool.tile([128, NT, 512], DT, tag="wq")
            wk_sb = pool.tile([128, NT, 512], DT, tag="wk")
            for c in range(4):
                nc.sync.dma_start(wq_sb[:, 4 * c : 4 * c + 4, :], wqTr[:, 4 * c : 4 * c + 4, :])
            for c in range(4):
                nc.sync.dma_start(wk_sb[:, 4 * c : 4 * c + 4, :], wkTr[:, 4 * c : 4 * c + 4, :])

    # ------------- phase B: Q then K projection + RoPE -------------------
    qr = [pool.tile([128, L], DT, tag=f"qr{h}", name=f"qr{h}") for h in range(GH)]
    kr = [pool.tile([128, L], DT, tag=f"kr{h}", name=f"kr{h}") for h in range(GH)]

    for w_sb, dst in ((wq_sb, qr), (wk_sb, kr)):
        for ic in range(NI):
            isl = slice(ic * 512, (ic + 1) * 512)
            pps = [ps.tile([128, 512], F32, tag="mm", name=f"pp{dt}") for dt in range(GH)]
            for mt in range(NT):
                xsl = load_slab(ic, mt)
                for dt in range(GH):
                    nc.tensor.matmul(
                        pps[dt][:],
                        w_sb[:, mt, dt * 128 : (dt + 1) * 128],
                        xsl[:],
                        start=(mt == 0),
                        stop=(mt == NT - 1),
                    )
            for dt in range(GH):
                pre = work.tile([128, 512], DT, tag="pre")
                nc.vector.tensor_copy(pre[:], pps[dt][:])
                rp = ps.tile([128, 512], F32, tag="acc", bufs=2)
                nc.tensor.matmul(rp[:], rot[:], pre[:], start=True, stop=True)
                t1 = work.tile([128, 512], F32, tag="t1")
                nc.vector.tensor_tensor(t1[:], pre[:], cos_sb[:, isl], MULT)
                t2 = work.tile([128, 512], F32, tag="t2")
                nc.vector.tensor_tensor(t2[:], rp[:], sin_sb[:, isl], MULT)
                nc.vector.tensor_tensor(dst[dt][:, isl], t1[:], t2[:], ADD)

    # ------------- phase C: causal attention per head --------------------
    ot = [pool.tile([128, L], DT, tag=f"ot{h}", name=f"ot{h}") for h in range(GH)]
    e_tiles = [pool.tile([128, 512], DT, tag=f"E{jt}", name=f"et{jt}") for jt in range(NT)]
    wo_sb = pool.tile([128, GH, L], DT, tag="wo")
    for h in range(GH):
        for I in range(NI):
            nj = (I + 1) * 4
            i0 = I * 512
            isl = slice(i0, i0 + 512)
            for jt in range(nj):
                st = ps.tile([128, 512], F32, tag="mm", name="st")
                nc.tensor.matmul(
                    st[:],
                    kr[h][:, jt * 128 : (jt + 1) * 128],
                    qr[h][:, isl],
                    start=True,
                    stop=True,
                )
                et = e_tiles[jt]
                nc.scalar.activation(
                    et[:], st[:], mybir.ActivationFunctionType.Exp,
                    scale=SCALE, bias=ebias[:],
                )
                if jt >= I * 4:
                    # keep where i >= j: (i0 + c) - (jt*128 + p) >= 0
                    nc.gpsimd.affine_select(
                        out=et[:],
                        in_=et[:],
                        compare_op=IS_GE,
                        fill=0.0,
                        base=i0 - jt * 128,
                        pattern=[[1, 512]],
                        channel_multiplier=-1,
                    )
            ov = ps.tile([128, 512], F32, tag="acc", bufs=2)
            for jt in range(nj):
                nc.tensor.matmul(
                    ov[:],
                    v_sb[jt][:, h * 128 : (h + 1) * 128],
                    e_tiles[jt][:],
                    start=(jt == 0),
                    stop=(jt == nj - 1),
                )
            dn = ps.tile([1, 512], F32, tag="dn", bufs=2)
            for jt in range(nj):
                nc.tensor.matmul(
                    dn[:], ones[:], e_tiles[jt][:], start=(jt == 0), stop=(jt == nj - 1)
                )
            den_sb = dpool.tile([1, 512], F32, tag="den")
            nc.vector.tensor_copy(den_sb[:], dn[:])
            rb = dpool.tile([128, 512], F32, tag="rb")
            nc.gpsimd.partition_broadcast(rb[:], den_sb[:], channels=128)
            rbi = dpool.tile([128, 512], F32, tag="rbi")
            nc.vector.reciprocal(rbi[:], rb[:])
            nc.vector.tensor_tensor(ot[h][:, isl], ov[:], rbi[:], MULT)
        if h == 0:
            # prefetch Wo behind attention compute
            for s_ in range(GH):
                nc.sync.dma_start(wo_sb[:, s_, :], woTr[:, s_, :])

    # ------------- phase D: partial output projection --------------------
    for it in range(NT):
        for fc in range(NI):
            op = ps.tile([128, 512], F32, tag="mm", name="op")
            for h in range(GH):
                nc.tensor.matmul(
                    op[:],
                    ot[h][:, it * 128 : (it + 1) * 128],
                    wo_sb[:, h, fc * 512 : (fc + 1) * 512],
                    start=(h == 0),
                    stop=(h == GH - 1),
                )
            ob = obpool.tile([128, 512], F32, tag="ob")
            if (it + fc) % 2 == 0:
                nc.vector.tensor_copy(ob[:], op[:])
            else:
                nc.scalar.copy(ob[:], op[:])
            nc.sync.dma_start(
                out[it * 128 : (it + 1) * 128, fc * 512 : (fc + 1) * 512], ob[:]
            )


def build():
    import contextlib

    nc = bacc.Bacc("TRN2", target_bir_lowering=False, debug=False, num_devices=NCORES)
    io = {
        "xT": nc.dram_tensor("xT", [HID, L], DT, kind="ExternalInput").ap(),
        "wqT": nc.dram_tensor("wqT", [HID, E], DT, kind="ExternalInput").ap(),
        "wkT": nc.dram_tensor("wkT", [HID, E], DT, kind="ExternalInput").ap(),
        "wvT": nc.dram_tensor("wvT", [HID, E], DT, kind="ExternalInput").ap(),
        "woT": nc.dram_tensor("woT", [E, HID], DT, kind="ExternalInput").ap(),
        "cosT": nc.dram_tensor("cosT", [D, L], F32, kind="ExternalInput").ap(),
        "sinT": nc.dram_tensor("sinT", [D, L], F32, kind="ExternalInput").ap(),
        "rotT": nc.dram_tensor("rotT", [D, D], DT, kind="ExternalInput").ap(),
        "out": nc.dram_tensor("out", [L, HID], F32, kind="ExternalOutput").ap(),
    }
    with tile.TileContext(nc) as tc:
        with contextlib.ExitStack() as ctx:
            _emit(nc, tc, ctx, io)
    nc.compile()
    return nc


_NC_CACHE = []


def _rot_matrix():
    # lhsT for the rotate_half matmul: rot(q) = P @ q, lhsT = P^T.
    rotT = np.zeros((D, D), dtype=NP_DT)
    for d in range(D // 2):
        rotT[d, d + 64] = 1.0
        rotT[d + 64, d] = -1.0
    return rotT


def make_in_maps(hidden_states, cos, sin, Wq, Wk, Wv, Wo):
    f = NP_DT
    cosT = np.ascontiguousarray(cos.T.astype(np.float32))
    sinT = np.ascontiguousarray(sin.T.astype(np.float32))
    rotT = _rot_matrix()
    xTs = [np.ascontiguousarray(hidden_states[b].T.astype(f)) for b in range(B)]
    in_maps = []
    for c in range(NCORES):
        b, g = divmod(c, 4)
        sl = slice(g * E, (g + 1) * E)
        in_maps.append({
            "xT": xTs[b],
            "wqT": np.ascontiguousarray(Wq[sl].T.astype(f)),
            "wkT": np.ascontiguousarray(Wk[sl].T.astype(f)),
            "wvT": np.ascontiguousarray(Wv[sl].T.astype(f)),
            "woT": np.ascontiguousarray(Wo[:, sl].T.astype(f)),
            "cosT": cosT,
            "sinT": sinT,
            "rotT": rotT,
        })
    return in_maps


def kernel(hidden_states, cos, sin, Wq, Wk, Wv, Wo):
    hidden_states, cos, sin, Wq, Wk, Wv, Wo = (
        np.asarray(a) for a in (hidden_states, cos, sin, Wq, Wk, Wv, Wo)
    )
    if not _NC_CACHE:
        _NC_CACHE.append(build())
    nc = _NC_CACHE[0]
    in_maps = make_in_maps(hidden_states, cos, sin, Wq, Wk, Wv, Wo)
    r = run_bass_kernel_spmd(nc, in_maps, list(range(NCORES)))
    out = np.empty((B, L, HID), np.float32)
    for b in range(B):
        acc = r.results[4 * b]["out"].astype(np.float32).copy()
        for g in range(1, 4):
            acc += r.results[4 * b + g]["out"]
        out[b] = acc
    return out


# revision 11
# speedup vs baseline: 1.4035x; 1.1265x over previous
"""Causal self-attention (B=2, L=2048, HID=2048, H=16, D=128) on 8 trn2 cores.

Sharding: core c -> (batch b = c//4, head-group g = c%4 of 4 heads).
Each core computes q/k/v projections for its 512 features from its batch,
RoPE, causal attention for its 4 heads, and a partial output projection
against its Wo column slice. Host sums the 4 partials per batch.

All matmuls run in float32r (RNE-to-11-mantissa-bit inputs, fp32 accumulate).
"""
import numpy as np

import concourse.mybir as mybir
import concourse.tile as tile
from concourse import bacc
from concourse.bass_utils import run_bass_kernel_spmd

B, L, HID, H = 2, 2048, 2048, 16
D = 128               # head dim
NCORES = 8
GH = 4                # heads per core
E = GH * D            # 512 per-core qkv features
NT = HID // 128       # 16 contraction tiles
NI = L // 512         # 4 i-chunks of 512
SCALE = 1.0 / float(np.sqrt(D))

F32 = mybir.dt.float32
F32R = mybir.dt.float32r
MULT = mybir.AluOpType.mult
ADD = mybir.AluOpType.add
IS_GE = mybir.AluOpType.is_ge
DT = mybir.dt.float16       # on-chip matmul dtype
NP_DT = np.float16
EXP_BIAS = -4.0             # exp(s*scale - 4): fp16 overflow headroom, cancels in softmax


def _emit(nc, tc, ctx, io):
    xT, wqT, wkT, wvT, woT, cosT, sinT, rotT, out = (
        io["xT"], io["wqT"], io["wkT"], io["wvT"], io["woT"],
        io["cosT"], io["sinT"], io["rotT"], io["out"],
    )
    xTr = xT.rearrange("(t p) i -> p t i", p=128)       # [128, 16, 2048]
    wqTr = wqT.rearrange("(t p) e -> p t e", p=128)     # [128, 16, 512]
    wkTr = wkT.rearrange("(t p) e -> p t e", p=128)
    wvTr = wvT.rearrange("(t p) e -> p t e", p=128)
    woTr = woT.rearrange("(s p) f -> p s f", p=128)     # [128, 4, 2048]

    pool = ctx.enter_context(tc.tile_pool(name="main", bufs=1))
    xpool = ctx.enter_context(tc.tile_pool(name="xsl", bufs=6))
    work = ctx.enter_context(tc.tile_pool(name="work", bufs=2))
    obpool = ctx.enter_context(tc.tile_pool(name="ob", bufs=3))
    dpool = ctx.enter_context(tc.tile_pool(name="dp", bufs=2))
    # single PSUM pool, exactly 8 banks: mm(4) + acc(2) + dn(2)
    ps = ctx.enter_context(tc.tile_pool(name="ps", bufs=4, space="PSUM"))

    def load_slab(ic, mt):
        xsl = xpool.tile([128, 512], DT, tag="xsl", name="xsl")
        nc.sync.dma_start(xsl[:], xTr[:, mt, ic * 512 : (ic + 1) * 512])
        return xsl

    # ---------------- phase A: V projection -> V[j, e] tiles -------------
    wv_sb = pool.tile([128, NT, 512], DT, tag="wv")
    for c in range(4):
        nc.sync.dma_start(wv_sb[:, 4 * c : 4 * c + 4, :], wvTr[:, 4 * c : 4 * c + 4, :])
    v_sb = [pool.tile([128, E], DT, tag=f"v{jt}", name=f"v{jt}") for jt in range(NT)]

    ones_f = pool.tile([128, 1], F32, tag="ones_f")
    nc.gpsimd.memset(ones_f[:], 1.0)
    ebias = pool.tile([128, 1], F32, tag="ebias")
    nc.gpsimd.memset(ebias[:], EXP_BIAS)
    ones = pool.tile([128, 1], DT, tag="ones")
    nc.vector.tensor_copy(ones[:], ones_f[:])
    rot = pool.tile([128, 128], DT, tag="rot")
    nc.sync.dma_start(rot[:], rotT)

    for icj in range(NI):
        vps = [ps.tile([128, 512], F32, tag="mm", name=f"vp{jt}") for jt in range(4)]
        for mt in range(NT):
            xsl = load_slab(icj, mt)
            for jt in range(4):
                nc.tensor.matmul(
                    vps[jt][:],
                    xsl[:, jt * 128 : (jt + 1) * 128],
                    wv_sb[:, mt, :],
                    start=(mt == 0),
                    stop=(mt == NT - 1),
                )
        for jt in range(4):
            nc.scalar.copy(v_sb[4 * icj + jt][:], vps[jt][:])
        if icj == 0:
            # prefetch q/k weights and rope tables behind phase A's compute
            cos_sb = pool.tile([128, L], F32, tag="cos")
            sin_sb = pool.tile([128, L], F32, tag="sin")
            nc.sync.dma_start(cos_sb[:], cosT)
            nc.sync.dma_start(sin_sb[:], sinT)
            wq_sb = pool.tile([128, NT, 512], DT, tag="wq")
            wk_sb = pool.tile([128, NT, 512], DT, tag="wk")
        if icj in (1, 2):
            c0 = 2 * (icj - 1)
            for c in (c0, c0 + 1):
                nc.sync.dma_start(wq_sb[:, 4 * c : 4 * c + 4, :], wqTr[:, 4 * c : 4 * c + 4, :])
                nc.sync.dma_start(wk_sb[:, 4 * c : 4 * c + 4, :], wkTr[:, 4 * c : 4 * c + 4, :])

    # ------------- phase B: Q then K projection + RoPE -------------------
    qr = [pool.tile([128, L], DT, tag=f"qr{h}", name=f"qr{h}") for h in range(GH)]
    kr = [pool.tile([128, L], DT, tag=f"kr{h}", name=f"kr{h}") for h in range(GH)]

    for ic in range(NI):
        for w_sb, dst in ((wq_sb, qr), (wk_sb, kr)):
            isl = slice(ic * 512, (ic + 1) * 512)
            pps = [ps.tile([128, 512], F32, tag="mm", name=f"pp{dt}") for dt in range(GH)]
            for mt in range(NT):
                xsl = load_slab(ic, mt)
                for dt in range(GH):
                    nc.tensor.matmul(
                        pps[dt][:],
                        w_sb[:, mt, dt * 128 : (dt + 1) * 128],
                        xsl[:],
                        start=(mt == 0),
                        stop=(mt == NT - 1),
                    )
            for dt in range(GH):
                pre = work.tile([128, 512], DT, tag="pre")
                if dt % 2 == 0:
                    nc.scalar.copy(pre[:], pps[dt][:])
                else:
                    nc.vector.tensor_copy(pre[:], pps[dt][:])
                rp = ps.tile([128, 512], F32, tag="acc", bufs=2)
                nc.tensor.matmul(rp[:], rot[:], pre[:], start=True, stop=True)
                t1 = work.tile([128, 512], F32, tag="t1")
                nc.vector.tensor_tensor(t1[:], pre[:], cos_sb[:, isl], MULT)
                t2 = work.tile([128, 512], F32, tag="t2")
                nc.vector.tensor_tensor(t2[:], rp[:], sin_sb[:, isl], MULT)
                nc.vector.tensor_tensor(dst[dt][:, isl], t1[:], t2[:], ADD)

    # ------------- phase C: causal attention per head --------------------
    ot = [pool.tile([128, L], DT, tag=f"ot{h}", name=f"ot{h}") for h in range(GH)]
    e_tiles = [pool.tile([128, 512], DT, tag=f"E{jt}", name=f"et{jt}") for jt in range(NT)]
    wo_sb = pool.tile([128, GH, L], DT, tag="wo")
    for h in range(GH):
        for I in range(NI):
            nj = (I + 1) * 4
            i0 = I * 512
            isl = slice(i0, i0 + 512)
            # diag tile jt = I*4 + t has valid columns [128*t, 512) only
            def vc0(jt):
                return max(0, (jt - I * 4) * 128)

            for jt in range(nj):
                c0_ = vc0(jt)
                st = ps.tile([128, 512], F32, tag="mm", name="st")
                nc.tensor.matmul(
                    st[:, c0_:],
                    kr[h][:, jt * 128 : (jt + 1) * 128],
                    qr[h][:, i0 + c0_ : i0 + 512],
                    start=True,
                    stop=True,
                )
                et = e_tiles[jt]
                nc.scalar.activation(
                    et[:, c0_:], st[:, c0_:], mybir.ActivationFunctionType.Exp,
                    scale=SCALE, bias=ebias[:],
                )
                if jt >= I * 4:
                    # within valid cols keep upper triangle: c' - p >= 0
                    nc.gpsimd.affine_select(
                        out=et[:, c0_:],
                        in_=et[:, c0_:],
                        compare_op=IS_GE,
                        fill=0.0,
                        base=0,
                        pattern=[[1, 512 - c0_]],
                        channel_multiplier=-1,
                    )
            ov = ps.tile([128, 512], F32, tag="acc", bufs=2)
            for jt in range(nj):
                c0_ = vc0(jt)
                nc.tensor.matmul(
                    ov[:, c0_:],
                    v_sb[jt][:, h * 128 : (h + 1) * 128],
                    e_tiles[jt][:, c0_:],
                    start=(jt == 0),
                    stop=(jt == nj - 1),
                )
            dn = ps.tile([1, 512], F32, tag="dn", bufs=2)
            for jt in range(nj):
                c0_ = vc0(jt)
                nc.tensor.matmul(
                    dn[:, c0_:], ones[:], e_tiles[jt][:, c0_:],
                    start=(jt == 0), stop=(jt == nj - 1)
                )
            den_sb = dpool.tile([1, 512], F32, tag="den")
            nc.vector.tensor_copy(den_sb[:], dn[:])
            rb = dpool.tile([128, 512], F32, tag="rb")
            nc.gpsimd.partition_broadcast(rb[:], den_sb[:], channels=128)
            rbi = dpool.tile([128, 512], F32, tag="rbi")
            nc.vector.reciprocal_approx_fast(out=rbi[:], in_=rb[:])
            nc.vector.tensor_tensor(ot[h][:, isl], ov[:], rbi[:], MULT)
        if h == 0:
            # prefetch Wo behind attention compute
            for s_ in range(GH):
                nc.sync.dma_start(wo_sb[:, s_, :], woTr[:, s_, :])

    # ------------- phase D: partial output projection --------------------
    dtags = [("mm", 4), ("mm", 4), ("acc", 2), ("dn", 2)]
    for it in range(NT):
        for fc in range(NI):
            dtag, dbufs = dtags[(it * NI + fc) % 4]
            op = ps.tile([128, 512], F32, tag=dtag, bufs=dbufs, name="op")
            for h in range(GH):
                nc.tensor.matmul(
                    op[:],
                    ot[h][:, it * 128 : (it + 1) * 128],
                    wo_sb[:, h, fc * 512 : (fc + 1) * 512],
                    start=(h == 0),
                    stop=(h == GH - 1),
                )
            ob = obpool.tile([128, 512], F32, tag="ob")
            if (it + fc) % 2 == 0:
                nc.vector.tensor_copy(ob[:], op[:])
            else:
                nc.scalar.copy(ob[:], op[:])

            nc.sync.dma_start(
                out[it * 128 : (it + 1) * 128, fc * 512 : (fc + 1) * 512], ob[:]
            )


def build():
    import contextlib

    nc = bacc.Bacc("TRN2", target_bir_lowering=False, debug=False, num_devices=NCORES)
    io = {
        "xT": nc.dram_tensor("xT", [HID, L], DT, kind="ExternalInput").ap(),
        "wqT": nc.dram_tensor("wqT", [HID, E], DT, kind="ExternalInput").ap(),
        "wkT": nc.dram_tensor("wkT", [HID, E], DT, kind="ExternalInput").ap(),
        "wvT": nc.dram_tensor("wvT", [HID, E], DT, kind="ExternalInput").ap(),
        "woT": nc.dram_tensor("woT", [E, HID], DT, kind="ExternalInput").ap(),
        "cosT": nc.dram_tensor("cosT", [D, L], F32, kind="ExternalInput").ap(),
        "sinT": nc.dram_tensor("sinT", [D, L], F32, kind="ExternalInput").ap(),
        "rotT": nc.dram_tensor("rotT", [D, D], DT, kind="ExternalInput").ap(),
        "out": nc.dram_tensor("out", [L, HID], F32, kind="ExternalOutput").ap(),
    }
    with tile.TileContext(nc) as tc:
        with contextlib.ExitStack() as ctx:
            _emit(nc, tc, ctx, io)
    nc.compile()
    return nc


_NC_CACHE = []


def _rot_matrix():
    # lhsT for the rotate_half matmul: rot(q) = P @ q, lhsT = P^T.
    rotT = np.zeros((D, D), dtype=NP_DT)
    for d in range(D // 2):
        rotT[d, d + 64] = 1.0
        rotT[d + 64, d] = -1.0
    return rotT


def make_in_maps(hidden_states, cos, sin, Wq, Wk, Wv, Wo):
    f = NP_DT
    cosT = np.ascontiguousarray(cos.T.astype(np.float32))
    sinT = np.ascontiguousarray(sin.T.astype(np.float32))
    rotT = _rot_matrix()
    xTs = [np.ascontiguousarray(hidden_states[b].T.astype(f)) for b in range(B)]
    in_maps = []
    for c in range(NCORES):
        b, g = divmod(c, 4)
        sl = slice(g * E, (g + 1) * E)
        in_maps.append({
            "xT": xTs[b],
            "wqT": np.ascontiguousarray(Wq[sl].T.astype(f)),
            "wkT": np.ascontiguousarray(Wk[sl].T.astype(f)),
            "wvT": np.ascontiguousarray(Wv[sl].T.astype(f)),
            "woT": np.ascontiguousarray(Wo[:, sl].T.astype(f)),
            "cosT": cosT,
            "sinT": sinT,
            "rotT": rotT,
        })
    return in_maps


def kernel(hidden_states, cos, sin, Wq, Wk, Wv, Wo):
    hidden_states, cos, sin, Wq, Wk, Wv, Wo = (
        np.asarray(a) for a in (hidden_states, cos, sin, Wq, Wk, Wv, Wo)
    )
    if not _NC_CACHE:
        _NC_CACHE.append(build())
    nc = _NC_CACHE[0]
    in_maps = make_in_maps(hidden_states, cos, sin, Wq, Wk, Wv, Wo)
    r = run_bass_kernel_spmd(nc, in_maps, list(range(NCORES)))
    out = np.empty((B, L, HID), np.float32)
    for b in range(B):
        acc = r.results[4 * b]["out"].astype(np.float32).copy()
        for g in range(1, 4):
            acc += r.results[4 * b + g]["out"]
        out[b] = acc
    return out


# revision 13
# speedup vs baseline: 1.5490x; 1.1037x over previous
"""Causal self-attention (B=2, L=2048, HID=2048, H=16, D=128) on 8 trn2 cores.

Sharding: core c -> (batch b = c//4, head-group g = c%4 of 4 heads).
Each core computes q/k/v projections for its 512 features from its batch,
RoPE, causal attention for its 4 heads, and a partial output projection
against its Wo column slice. Host sums the 4 partials per batch.

All matmuls run in float32r (RNE-to-11-mantissa-bit inputs, fp32 accumulate).
"""
import numpy as np

import concourse.mybir as mybir
import concourse.tile as tile
from concourse import bacc
from concourse.bass_utils import run_bass_kernel_spmd

B, L, HID, H = 2, 2048, 2048, 16
D = 128               # head dim
NCORES = 8
GH = 4                # heads per core
E = GH * D            # 512 per-core qkv features
NT = HID // 128       # 16 contraction tiles
NI = L // 512         # 4 i-chunks of 512
SCALE = 1.0 / float(np.sqrt(D))

F32 = mybir.dt.float32
F32R = mybir.dt.float32r
MULT = mybir.AluOpType.mult
ADD = mybir.AluOpType.add
IS_GE = mybir.AluOpType.is_ge
DT = mybir.dt.float16       # on-chip matmul dtype
NP_DT = np.float16
EXP_BIAS = -4.0             # exp(s*scale - 4): fp16 overflow headroom, cancels in softmax


def _emit(nc, tc, ctx, io):
    xT, wqT, wkT, wvT, woT, cosT, sinT, rotT, out = (
        io["xT"], io["wqT"], io["wkT"], io["wvT"], io["woT"],
        io["cosT"], io["sinT"], io["rotT"], io["out"],
    )
    xTr = xT.rearrange("(t p) i -> p t i", p=128)       # [128, 16, 2048]
    wqTr = wqT.rearrange("(t p) e -> p t e", p=128)     # [128, 16, 512]
    wkTr = wkT.rearrange("(t p) e -> p t e", p=128)
    wvTr = wvT.rearrange("(t p) e -> p t e", p=128)
    woTr = woT.rearrange("(s p) f -> p s f", p=128)     # [128, 4, 2048]

    pool = ctx.enter_context(tc.tile_pool(name="main", bufs=1))
    xpool = ctx.enter_context(tc.tile_pool(name="xsl", bufs=8))
    work = ctx.enter_context(tc.tile_pool(name="work", bufs=2))
    obpool = ctx.enter_context(tc.tile_pool(name="ob", bufs=3))
    dpool = ctx.enter_context(tc.tile_pool(name="dp", bufs=2))
    # single PSUM pool, exactly 8 banks: mm(4) + acc(2) + dn(2)
    ps = ctx.enter_context(tc.tile_pool(name="ps", bufs=4, space="PSUM"))

    def load_slab(ic, mt):
        xsl = xpool.tile([128, 512], DT, tag="xsl", name="xsl")
        nc.sync.dma_start(xsl[:], xTr[:, mt, ic * 512 : (ic + 1) * 512])
        return xsl

    # ---------------- phase A: V projection -> V[j, e] tiles -------------
    wv_sb = pool.tile([128, NT, 512], DT, tag="wv")
    for c in range(4):
        nc.sync.dma_start(wv_sb[:, 4 * c : 4 * c + 4, :], wvTr[:, 4 * c : 4 * c + 4, :])
    v_sb = [pool.tile([128, E], DT, tag=f"v{jt}", name=f"v{jt}") for jt in range(NT)]

    ones_f = pool.tile([128, 1], F32, tag="ones_f")
    nc.gpsimd.memset(ones_f[:], 1.0)
    ebias = pool.tile([128, 1], F32, tag="ebias")
    nc.gpsimd.memset(ebias[:], EXP_BIAS)
    ones = pool.tile([128, 1], DT, tag="ones")
    nc.vector.tensor_copy(ones[:], ones_f[:])
    rot = pool.tile([128, 128], DT, tag="rot")
    nc.sync.dma_start(rot[:], rotT)

    for icj in range(NI):
        vps = [ps.tile([128, 512], F32, tag="mm", name=f"vp{jt}") for jt in range(4)]
        for mt in range(NT):
            xsl = load_slab(icj, mt)
            for jt in range(4):
                nc.tensor.matmul(
                    vps[jt][:],
                    xsl[:, jt * 128 : (jt + 1) * 128],
                    wv_sb[:, mt, :],
                    start=(mt == 0),
                    stop=(mt == NT - 1),
                )
        for jt in range(4):
            nc.scalar.copy(v_sb[4 * icj + jt][:], vps[jt][:])
        if icj == 0:
            # prefetch q/k weights and rope tables behind phase A's compute
            cos_sb = pool.tile([128, L], F32, tag="cos")
            sin_sb = pool.tile([128, L], F32, tag="sin")
            nc.sync.dma_start(cos_sb[:], cosT)
            nc.sync.dma_start(sin_sb[:], sinT)
            wq_sb = pool.tile([128, NT, 512], DT, tag="wq")
            wk_sb = pool.tile([128, NT, 512], DT, tag="wk")
        if icj in (1, 2):
            c0 = 2 * (icj - 1)
            for c in (c0, c0 + 1):
                nc.sync.dma_start(wq_sb[:, 4 * c : 4 * c + 4, :], wqTr[:, 4 * c : 4 * c + 4, :])
                nc.sync.dma_start(wk_sb[:, 4 * c : 4 * c + 4, :], wkTr[:, 4 * c : 4 * c + 4, :])

    # ------------- phase B: Q then K projection + RoPE -------------------
    qr = [pool.tile([128, L], DT, tag=f"qr{h}", name=f"qr{h}") for h in range(GH)]
    kr = [pool.tile([128, L], DT, tag=f"kr{h}", name=f"kr{h}") for h in range(GH)]

    pending = []

    def emit_rope(batch):
        for pre, dst, dt, isl_ in batch:
            rp = ps.tile([128, 512], F32, tag="acc", bufs=2)
            nc.tensor.matmul(rp[:], rot[:], pre[:], start=True, stop=True)
            t1 = work.tile([128, 512], F32, tag="t1")
            nc.vector.tensor_tensor(t1[:], pre[:], cos_sb[:, isl_], MULT)
            t2 = work.tile([128, 512], F32, tag="t2")
            nc.vector.tensor_tensor(t2[:], rp[:], sin_sb[:, isl_], MULT)
            nc.vector.tensor_tensor(dst[dt][:, isl_], t1[:], t2[:], ADD)

    for ic in range(NI):
        for w_sb, dst in ((wq_sb, qr), (wk_sb, kr)):
            isl = slice(ic * 512, (ic + 1) * 512)
            pps = [ps.tile([128, 512], F32, tag="mm", name=f"pp{dt}") for dt in range(GH)]
            for mt in range(NT):
                xsl = load_slab(ic, mt)
                for dt in range(GH):
                    nc.tensor.matmul(
                        pps[dt][:],
                        w_sb[:, mt, dt * 128 : (dt + 1) * 128],
                        xsl[:],
                        start=(mt == 0),
                        stop=(mt == NT - 1),
                    )
            batch = []
            for dt in range(GH):
                pre = work.tile([128, 512], DT, tag="pre", bufs=6)
                if dt % 2 == 0:
                    nc.scalar.copy(pre[:], pps[dt][:])
                else:
                    nc.vector.tensor_copy(pre[:], pps[dt][:])
                batch.append((pre, dst, dt, isl))
            # rope for the PREVIOUS group: its pre tiles are long since ready,
            # so the rot matmuls never stall PE at the group boundary
            if pending:
                emit_rope(pending.pop())
            pending.append(batch)
    while pending:
        emit_rope(pending.pop())

    # ------------- phase C: causal attention per head --------------------
    ot = [pool.tile([128, L], DT, tag=f"ot{h}", name=f"ot{h}") for h in range(GH)]
    e_tiles = [pool.tile([128, 512], DT, tag=f"E{jt}", name=f"et{jt}") for jt in range(NT)]
    wo_sb = pool.tile([128, GH, L], DT, tag="wo")
    for s_ in range(GH):
        nc.sync.dma_start(wo_sb[:, s_, :], woTr[:, s_, :])
    for I in range(NI):
        for h in range(GH):
            nj = (I + 1) * 4
            i0 = I * 512
            isl = slice(i0, i0 + 512)
            # diag tile jt = I*4 + t has valid columns [128*t, 512) only
            def vc0(jt):
                return max(0, (jt - I * 4) * 128)

            for jt in range(nj):
                c0_ = vc0(jt)
                st = ps.tile([128, 512], F32, tag="mm", name="st")
                nc.tensor.matmul(
                    st[:, c0_:],
                    kr[h][:, jt * 128 : (jt + 1) * 128],
                    qr[h][:, i0 + c0_ : i0 + 512],
                    start=True,
                    stop=True,
                )
                et = e_tiles[jt]
                nc.scalar.activation(
                    et[:, c0_:], st[:, c0_:], mybir.ActivationFunctionType.Exp,
                    scale=SCALE, bias=ebias[:],
                )
                if jt >= I * 4:
                    # within valid cols keep upper triangle: c' - p >= 0
                    nc.gpsimd.affine_select(
                        out=et[:, c0_:],
                        in_=et[:, c0_:],
                        compare_op=IS_GE,
                        fill=0.0,
                        base=0,
                        pattern=[[1, 512 - c0_]],
                        channel_multiplier=-1,
                    )
            ov = ps.tile([128, 512], F32, tag="acc", bufs=2)
            for jt in range(nj):
                c0_ = vc0(jt)
                nc.tensor.matmul(
                    ov[:, c0_:],
                    v_sb[jt][:, h * 128 : (h + 1) * 128],
                    e_tiles[jt][:, c0_:],
                    start=(jt == 0),
                    stop=(jt == nj - 1),
                )
            dn = ps.tile([1, 512], F32, tag="dn", bufs=2)
            for jt in range(nj):
                c0_ = vc0(jt)
                nc.tensor.matmul(
                    dn[:, c0_:], ones[:], e_tiles[jt][:, c0_:],
                    start=(jt == 0), stop=(jt == nj - 1)
                )
            den_sb = dpool.tile([1, 512], F32, tag="den")
            nc.vector.tensor_copy(den_sb[:], dn[:])
            rb = dpool.tile([128, 512], F32, tag="rb")
            nc.gpsimd.partition_broadcast(rb[:], den_sb[:], channels=128)
            rbi = dpool.tile([128, 512], F32, tag="rbi")
            nc.vector.reciprocal_approx_fast(out=rbi[:], in_=rb[:])
            nc.vector.tensor_tensor(ot[h][:, isl], ov[:], rbi[:], MULT)
        # phase D segment: Wo blocks for query tiles completed by this I
        dtags = [("mm", 4), ("mm", 4), ("acc", 2), ("dn", 2)]
        for it in range(I * 4, I * 4 + 4):
            for fc in range(NI):
                dtag, dbufs = dtags[(it * NI + fc) % 4]
                op = ps.tile([128, 512], F32, tag=dtag, bufs=dbufs, name="op")
                for h in range(GH):
                    nc.tensor.matmul(
                        op[:],
                        ot[h][:, it * 128 : (it + 1) * 128],
                        wo_sb[:, h, fc * 512 : (fc + 1) * 512],
                        start=(h == 0),
                        stop=(h == GH - 1),
                    )
                ob = obpool.tile([128, 512], F32, tag="ob")
                if (it + fc) % 2 == 0:
                    nc.vector.tensor_copy(ob[:], op[:])
                else:
                    nc.scalar.copy(ob[:], op[:])
                nc.sync.dma_start(
                    out[it * 128 : (it + 1) * 128, fc * 512 : (fc + 1) * 512], ob[:]
                )



def build():
    import contextlib

    nc = bacc.Bacc("TRN2", target_bir_lowering=False, debug=False, num_devices=NCORES)
    io = {
        "xT": nc.dram_tensor("xT", [HID, L], DT, kind="ExternalInput").ap(),
        "wqT": nc.dram_tensor("wqT", [HID, E], DT, kind="ExternalInput").ap(),
        "wkT": nc.dram_tensor("wkT", [HID, E], DT, kind="ExternalInput").ap(),
        "wvT": nc.dram_tensor("wvT", [HID, E], DT, kind="ExternalInput").ap(),
        "woT": nc.dram_tensor("woT", [E, HID], DT, kind="ExternalInput").ap(),
        "cosT": nc.dram_tensor("cosT", [D, L], F32, kind="ExternalInput").ap(),
        "sinT": nc.dram_tensor("sinT", [D, L], F32, kind="ExternalInput").ap(),
        "rotT": nc.dram_tensor("rotT", [D, D], DT, kind="ExternalInput").ap(),
        "out": nc.dram_tensor("out", [L, HID], F32, kind="ExternalOutput").ap(),
    }
    with tile.TileContext(nc) as tc:
        with contextlib.ExitStack() as ctx:
            _emit(nc, tc, ctx, io)
    nc.compile()
    return nc


_NC_CACHE = []


def _rot_matrix():
    # lhsT for the rotate_half matmul: rot(q) = P @ q, lhsT = P^T.
    rotT = np.zeros((D, D), dtype=NP_DT)
    for d in range(D // 2):
        rotT[d, d + 64] = 1.0
        rotT[d + 64, d] = -1.0
    return rotT


def make_in_maps(hidden_states, cos, sin, Wq, Wk, Wv, Wo):
    f = NP_DT
    cosT = np.ascontiguousarray(cos.T.astype(np.float32))
    sinT = np.ascontiguousarray(sin.T.astype(np.float32))
    rotT = _rot_matrix()
    xTs = [np.ascontiguousarray(hidden_states[b].T.astype(f)) for b in range(B)]
    in_maps = []
    for c in range(NCORES):
        b, g = divmod(c, 4)
        sl = slice(g * E, (g + 1) * E)
        in_maps.append({
            "xT": xTs[b],
            "wqT": np.ascontiguousarray(Wq[sl].T.astype(f)),
            "wkT": np.ascontiguousarray(Wk[sl].T.astype(f)),
            "wvT": np.ascontiguousarray(Wv[sl].T.astype(f)),
            "woT": np.ascontiguousarray(Wo[:, sl].T.astype(f)),
            "cosT": cosT,
            "sinT": sinT,
            "rotT": rotT,
        })
    return in_maps


def kernel(hidden_states, cos, sin, Wq, Wk, Wv, Wo):
    hidden_states, cos, sin, Wq, Wk, Wv, Wo = (
        np.asarray(a) for a in (hidden_states, cos, sin, Wq, Wk, Wv, Wo)
    )
    if not _NC_CACHE:
        _NC_CACHE.append(build())
    nc = _NC_CACHE[0]
    in_maps = make_in_maps(hidden_states, cos, sin, Wq, Wk, Wv, Wo)
    r = run_bass_kernel_spmd(nc, in_maps, list(range(NCORES)))
    out = np.empty((B, L, HID), np.float32)
    for b in range(B):
        acc = r.results[4 * b]["out"].astype(np.float32).copy()
        for g in range(1, 4):
            acc += r.results[4 * b + g]["out"]
        out[b] = acc
    return out


# revision 16
# speedup vs baseline: 1.5692x; 1.0131x over previous
"""Causal self-attention (B=2, L=2048, HID=2048, H=16, D=128) on 8 trn2 cores.

Sharding: core c -> (batch b = c//4, head-group g = c%4 of 4 heads).
Each core computes q/k/v projections for its 512 features from its batch,
RoPE, causal attention for its 4 heads, and a partial output projection
against its Wo column slice. Host sums the 4 partials per batch.

All matmuls run in float32r (RNE-to-11-mantissa-bit inputs, fp32 accumulate).
"""
import numpy as np

import concourse.mybir as mybir
import concourse.tile as tile
from concourse import bacc
from concourse.bass_utils import run_bass_kernel_spmd

B, L, HID, H = 2, 2048, 2048, 16
D = 128               # head dim
NCORES = 8
GH = 4                # heads per core
E = GH * D            # 512 per-core qkv features
NT = HID // 128       # 16 contraction tiles
NI = L // 512         # 4 i-chunks of 512
SCALE = 1.0 / float(np.sqrt(D))

F32 = mybir.dt.float32
F32R = mybir.dt.float32r
MULT = mybir.AluOpType.mult
ADD = mybir.AluOpType.add
IS_GE = mybir.AluOpType.is_ge
DT = mybir.dt.float16       # on-chip matmul dtype
NP_DT = np.float16
EXP_BIAS = -4.0             # exp(s*scale - 4): fp16 overflow headroom, cancels in softmax


def _emit(nc, tc, ctx, io):
    xT, wqT, wkT, wvT, woT, cosT, sinT, rotT, out = (
        io["xT"], io["wqT"], io["wkT"], io["wvT"], io["woT"],
        io["cosT"], io["sinT"], io["rotT"], io["out"],
    )
    xTr = xT.rearrange("(t p) i -> p t i", p=128)       # [128, 16, 2048]
    wqTr = wqT.rearrange("(t p) e -> p t e", p=128)     # [128, 16, 512]
    wkTr = wkT.rearrange("(t p) e -> p t e", p=128)
    wvTr = wvT.rearrange("(t p) e -> p t e", p=128)
    woTr = woT.rearrange("(s p) f -> p s f", p=128)     # [128, 4, 2048]

    pool = ctx.enter_context(tc.tile_pool(name="main", bufs=1))
    xpool = ctx.enter_context(tc.tile_pool(name="xsl", bufs=4))
    work = ctx.enter_context(tc.tile_pool(name="work", bufs=2))
    obpool = ctx.enter_context(tc.tile_pool(name="ob", bufs=3))
    dpool = ctx.enter_context(tc.tile_pool(name="dp", bufs=1))
    # single PSUM pool, exactly 8 banks: mm(4) + acc(2) + dn(2)
    ps = ctx.enter_context(tc.tile_pool(name="ps", bufs=4, space="PSUM"))

    def load_quad(ic, g):
        """mt tiles 4g..4g+3 of xT[:, ic*512:+512] in one DMA."""
        xq = xpool.tile([128, 4, 512], DT, tag="xsl", name="xq")
        nc.sync.dma_start(xq[:], xTr[:, 4 * g : 4 * g + 4, ic * 512 : (ic + 1) * 512])
        return xq

    # ---------------- phase A: V projection -> V[j, e] tiles -------------
    wv_sb = pool.tile([128, NT, 512], DT, tag="wv")
    for c in range(4):
        nc.sync.dma_start(wv_sb[:, 4 * c : 4 * c + 4, :], wvTr[:, 4 * c : 4 * c + 4, :])
    v_sb = [pool.tile([128, E], DT, tag=f"v{jt}", name=f"v{jt}") for jt in range(NT)]

    ones_f = pool.tile([128, 1], F32, tag="ones_f")
    nc.gpsimd.memset(ones_f[:], 1.0)
    ebias = pool.tile([128, 1], F32, tag="ebias")
    nc.gpsimd.memset(ebias[:], EXP_BIAS)
    ones = pool.tile([128, 1], DT, tag="ones")
    nc.vector.tensor_copy(ones[:], ones_f[:])
    rot = pool.tile([128, 128], DT, tag="rot")
    nc.sync.dma_start(rot[:], rotT)

    for icj in range(NI):
        vps = [ps.tile([128, 512], F32, tag="mm", name=f"vp{jt}") for jt in range(4)]
        for mt in range(NT):
            if mt % 4 == 0:
                xq = load_quad(icj, mt // 4)
            for jt in range(4):
                nc.tensor.matmul(
                    vps[jt][:],
                    xq[:, mt % 4, jt * 128 : (jt + 1) * 128],
                    wv_sb[:, mt, :],
                    start=(mt == 0),
                    stop=(mt == NT - 1),
                )
        for jt in range(4):
            nc.scalar.copy(v_sb[4 * icj + jt][:], vps[jt][:])
        if icj == 0:
            # prefetch q/k weights and rope tables behind phase A's compute
            cos_sb = pool.tile([128, L], F32, tag="cos")
            sin_sb = pool.tile([128, L], F32, tag="sin")
            nc.sync.dma_start(cos_sb[:], cosT)
            nc.sync.dma_start(sin_sb[:], sinT)
            wq_sb = pool.tile([128, NT, 512], DT, tag="wq")
            wk_sb = pool.tile([128, NT, 512], DT, tag="wk")
        if icj in (1, 2):
            c0 = 2 * (icj - 1)
            for c in (c0, c0 + 1):
                nc.sync.dma_start(wq_sb[:, 4 * c : 4 * c + 4, :], wqTr[:, 4 * c : 4 * c + 4, :])
                nc.sync.dma_start(wk_sb[:, 4 * c : 4 * c + 4, :], wkTr[:, 4 * c : 4 * c + 4, :])

    # ------------- phase B: Q then K projection + RoPE -------------------
    qr = [pool.tile([128, L], DT, tag=f"qr{h}", name=f"qr{h}") for h in range(GH)]
    kr = [pool.tile([128, L], DT, tag=f"kr{h}", name=f"kr{h}") for h in range(GH)]

    pending = []

    def emit_rope(batch):
        for pre, dst, dt, isl_ in batch:
            rp = ps.tile([128, 512], F32, tag="acc", bufs=2)
            nc.tensor.matmul(rp[:], rot[:], pre[:], start=True, stop=True)
            t1 = work.tile([128, 512], F32, tag="t1")
            nc.vector.tensor_tensor(t1[:], pre[:], cos_sb[:, isl_], MULT)
            t2 = work.tile([128, 512], F32, tag="t2")
            nc.vector.tensor_tensor(t2[:], rp[:], sin_sb[:, isl_], MULT)
            nc.vector.tensor_tensor(dst[dt][:, isl_], t1[:], t2[:], ADD)

    for ic in range(NI):
        for w_sb, dst in ((wq_sb, qr), (wk_sb, kr)):
            isl = slice(ic * 512, (ic + 1) * 512)
            pps = [ps.tile([128, 512], F32, tag="mm", name=f"pp{dt}") for dt in range(GH)]
            for mt in range(NT):
                if mt % 4 == 0:
                    xq = load_quad(ic, mt // 4)
                for dt in range(GH):
                    nc.tensor.matmul(
                        pps[dt][:],
                        w_sb[:, mt, dt * 128 : (dt + 1) * 128],
                        xq[:, mt % 4, :],
                        start=(mt == 0),
                        stop=(mt == NT - 1),
                    )
            batch = []
            for dt in range(GH):
                pre = work.tile([128, 512], DT, tag="pre", bufs=4)
                if dt % 2 == 0:
                    nc.scalar.copy(pre[:], pps[dt][:])
                else:
                    nc.vector.tensor_copy(pre[:], pps[dt][:])
                batch.append((pre, dst, dt, isl))
            # rope for the PREVIOUS group: its pre tiles are long since ready,
            # so the rot matmuls never stall PE at the group boundary
            if pending:
                emit_rope(pending.pop())
            pending.append(batch)
    while pending:
        emit_rope(pending.pop())

    # ------------- phase C: causal attention per head --------------------
    ot = [pool.tile([128, L], DT, tag=f"ot{h}", name=f"ot{h}") for h in range(GH)]
    e_tiles = [pool.tile([128, 512], DT, tag=f"E{jt}", name=f"et{jt}") for jt in range(NT)]
    wo_sb = pool.tile([128, GH, L], DT, tag="wo")
    for s_ in range(GH):
        nc.sync.dma_start(wo_sb[:, s_, :], woTr[:, s_, :])
    for I in range(NI):
        for h in range(GH):
            nj = (I + 1) * 4
            i0 = I * 512
            isl = slice(i0, i0 + 512)
            # diag tile jt = I*4 + t has valid columns [128*t, 512) only
            def vc0(jt):
                return max(0, (jt - I * 4) * 128)

            for jt in range(nj):
                c0_ = vc0(jt)
                st = ps.tile([128, 512], F32, tag="mm", name="st")
                nc.tensor.matmul(
                    st[:, c0_:],
                    kr[h][:, jt * 128 : (jt + 1) * 128],
                    qr[h][:, i0 + c0_ : i0 + 512],
                    start=True,
                    stop=True,
                )
                et = e_tiles[jt]
                nc.scalar.activation(
                    et[:, c0_:], st[:, c0_:], mybir.ActivationFunctionType.Exp,
                    scale=SCALE, bias=ebias[:],
                )
                if jt >= I * 4:
                    # within valid cols keep upper triangle: c' - p >= 0
                    nc.gpsimd.affine_select(
                        out=et[:, c0_:],
                        in_=et[:, c0_:],
                        compare_op=IS_GE,
                        fill=0.0,
                        base=0,
                        pattern=[[1, 512 - c0_]],
                        channel_multiplier=-1,
                    )
            ovtag, dntag = ("dn", "acc") if h % 2 == 0 else ("acc", "dn")
            ov = ps.tile([128, 512], F32, tag=ovtag, bufs=2)
            for jt in range(nj):
                c0_ = vc0(jt)
                nc.tensor.matmul(
                    ov[:, c0_:],
                    v_sb[jt][:, h * 128 : (h + 1) * 128],
                    e_tiles[jt][:, c0_:],
                    start=(jt == 0),
                    stop=(jt == nj - 1),
                )
            dn = ps.tile([1, 512], F32, tag=dntag, bufs=2)
            for jt in range(nj):
                c0_ = vc0(jt)
                nc.tensor.matmul(
                    dn[:, c0_:], ones[:], e_tiles[jt][:, c0_:],
                    start=(jt == 0), stop=(jt == nj - 1)
                )
            den_sb = dpool.tile([1, 512], F32, tag="den")
            nc.vector.tensor_copy(den_sb[:], dn[:])
            rb = dpool.tile([128, 512], F32, tag="rb")
            nc.gpsimd.partition_broadcast(rb[:], den_sb[:], channels=128)
            rbi = dpool.tile([128, 512], F32, tag="rbi")
            nc.vector.reciprocal_approx_fast(out=rbi[:], in_=rb[:])
            nc.vector.tensor_tensor(ot[h][:, isl], ov[:], rbi[:], MULT)
        # phase D segment: Wo blocks for query tiles completed by this I
        dtags = [("mm", 4), ("mm", 4), ("acc", 2), ("dn", 2)]
        for it in range(I * 4, I * 4 + 4):
            for fp in range(2):  # fc pairs
                ob = obpool.tile([128, 1024], F32, tag="ob", bufs=2)
                for half in range(2):
                    fc = 2 * fp + half
                    dtag, dbufs = dtags[(it * NI + fc) % 4]
                    op = ps.tile([128, 512], F32, tag=dtag, bufs=dbufs, name="op")
                    for h in range(GH):
                        nc.tensor.matmul(
                            op[:],
                            ot[h][:, it * 128 : (it + 1) * 128],
                            wo_sb[:, h, fc * 512 : (fc + 1) * 512],
                            start=(h == 0),
                            stop=(h == GH - 1),
                        )
                    if (it + fc) % 2 == 0:
                        nc.vector.tensor_copy(ob[:, half * 512 : (half + 1) * 512], op[:])
                    else:
                        nc.scalar.copy(ob[:, half * 512 : (half + 1) * 512], op[:])
                nc.sync.dma_start(
                    out[it * 128 : (it + 1) * 128, fp * 1024 : (fp + 1) * 1024], ob[:]
                )



def build():
    import contextlib

    nc = bacc.Bacc("TRN2", target_bir_lowering=False, debug=False, num_devices=NCORES)
    io = {
        "xT": nc.dram_tensor("xT", [HID, L], DT, kind="ExternalInput").ap(),
        "wqT": nc.dram_tensor("wqT", [HID, E], DT, kind="ExternalInput").ap(),
        "wkT": nc.dram_tensor("wkT", [HID, E], DT, kind="ExternalInput").ap(),
        "wvT": nc.dram_tensor("wvT", [HID, E], DT, kind="ExternalInput").ap(),
        "woT": nc.dram_tensor("woT", [E, HID], DT, kind="ExternalInput").ap(),
        "cosT": nc.dram_tensor("cosT", [D, L], F32, kind="ExternalInput").ap(),
        "sinT": nc.dram_tensor("sinT", [D, L], F32, kind="ExternalInput").ap(),
        "rotT": nc.dram_tensor("rotT", [D, D], DT, kind="ExternalInput").ap(),
        "out": nc.dram_tensor("out", [L, HID], F32, kind="ExternalOutput").ap(),
    }
    with tile.TileContext(nc) as tc:
        with contextlib.ExitStack() as ctx:
            _emit(nc, tc, ctx, io)
    nc.compile()
    return nc


_NC_CACHE = []


def _rot_matrix():
    # lhsT for the rotate_half matmul: rot(q) = P @ q, lhsT = P^T.
    rotT = np.zeros((D, D), dtype=NP_DT)
    for d in range(D // 2):
        rotT[d, d + 64] = 1.0
        rotT[d + 64, d] = -1.0
    return rotT


def make_in_maps(hidden_states, cos, sin, Wq, Wk, Wv, Wo):
    f = NP_DT
    cosT = np.ascontiguousarray(cos.T.astype(np.float32))
    sinT = np.ascontiguousarray(sin.T.astype(np.float32))
    rotT = _rot_matrix()
    xTs = [np.ascontiguousarray(hidden_states[b].T.astype(f)) for b in range(B)]
    in_maps = []
    for c in range(NCORES):
        b, g = divmod(c, 4)
        sl = slice(g * E, (g + 1) * E)
        in_maps.append({
            "xT": xTs[b],
            "wqT": np.ascontiguousarray(Wq[sl].T.astype(f)),
            "wkT": np.ascontiguousarray(Wk[sl].T.astype(f)),
            "wvT": np.ascontiguousarray(Wv[sl].T.astype(f)),
            "woT": np.ascontiguousarray(Wo[:, sl].T.astype(f)),
            "cosT": cosT,
            "sinT": sinT,
            "rotT": rotT,
        })
    return in_maps


def kernel(hidden_states, cos, sin, Wq, Wk, Wv, Wo):
    hidden_states, cos, sin, Wq, Wk, Wv, Wo = (
        np.asarray(a) for a in (hidden_states, cos, sin, Wq, Wk, Wv, Wo)
    )
    if not _NC_CACHE:
        _NC_CACHE.append(build())
    nc = _NC_CACHE[0]
    in_maps = make_in_maps(hidden_states, cos, sin, Wq, Wk, Wv, Wo)
    r = run_bass_kernel_spmd(nc, in_maps, list(range(NCORES)))
    out = np.empty((B, L, HID), np.float32)
    for b in range(B):
        acc = r.results[4 * b]["out"].astype(np.float32).copy()
        for g in range(1, 4):
            acc += r.results[4 * b + g]["out"]
        out[b] = acc
    return out


# revision 18
# speedup vs baseline: 1.6627x; 1.0596x over previous
"""Causal self-attention (B=2, L=2048, HID=2048, H=16, D=128) on 8 trn2 cores.

Sharding: core c -> (batch b = c//4, head-group g = c%4 of 4 heads).
Each core computes q/k/v projections for its 512 features from its batch,
RoPE, causal attention for its 4 heads, and a partial output projection
against its Wo column slice. Host sums the 4 partials per batch.

All matmuls run in float32r (RNE-to-11-mantissa-bit inputs, fp32 accumulate).
"""
import numpy as np

import concourse.mybir as mybir
import concourse.tile as tile
from concourse import bacc
from concourse.bass_utils import run_bass_kernel_spmd

B, L, HID, H = 2, 2048, 2048, 16
D = 128               # head dim
NCORES = 8
GH = 4                # heads per core
E = GH * D            # 512 per-core qkv features
NT = HID // 128       # 16 contraction tiles
NI = L // 512         # 4 i-chunks of 512
SCALE = 1.0 / float(np.sqrt(D))

F32 = mybir.dt.float32
F32R = mybir.dt.float32r
MULT = mybir.AluOpType.mult
ADD = mybir.AluOpType.add
IS_GE = mybir.AluOpType.is_ge
DT = mybir.dt.float16       # on-chip matmul dtype
NP_DT = np.float16
EXP_BIAS = -4.0             # exp(s*scale - 4): fp16 overflow headroom, cancels in softmax


def _emit(nc, tc, ctx, io):
    xT, wqT, wkT, wvT, woT, cosT, sinT, rotT, out = (
        io["xT"], io["wqT"], io["wkT"], io["wvT"], io["woT"],
        io["cosT"], io["sinT"], io["rotT"], io["out"],
    )
    xTr = xT.rearrange("(t p) i -> p t i", p=128)       # [128, 16, 2048]
    wqTr = wqT.rearrange("(t p) e -> p t e", p=128)     # [128, 16, 512]
    wkTr = wkT.rearrange("(t p) e -> p t e", p=128)
    wvTr = wvT.rearrange("(t p) e -> p t e", p=128)
    woTr = woT.rearrange("(s p) f -> p s f", p=128)     # [128, 4, 2048]

    pool = ctx.enter_context(tc.tile_pool(name="main", bufs=1))
    xpool = ctx.enter_context(tc.tile_pool(name="xsl", bufs=4))
    work = ctx.enter_context(tc.tile_pool(name="work", bufs=2))
    obpool = ctx.enter_context(tc.tile_pool(name="ob", bufs=3))
    dpool = ctx.enter_context(tc.tile_pool(name="dp", bufs=1))
    # single PSUM pool, exactly 8 banks: mm(4) + acc(2) + dn(2)
    ps = ctx.enter_context(tc.tile_pool(name="ps", bufs=4, space="PSUM"))

    def load_quad(ic, g):
        """mt tiles 4g..4g+3 of xT[:, ic*512:+512] in one DMA."""
        xq = xpool.tile([128, 4, 512], DT, tag="xsl", name="xq")
        nc.sync.dma_start(xq[:], xTr[:, 4 * g : 4 * g + 4, ic * 512 : (ic + 1) * 512])
        return xq

    # ---------------- phase A: V projection -> V[j, e] tiles -------------
    first_xq = load_quad(0, 0)
    wv_sb = pool.tile([128, NT, 512], DT, tag="wv")
    for c in range(4):
        nc.sync.dma_start(wv_sb[:, 4 * c : 4 * c + 4, :], wvTr[:, 4 * c : 4 * c + 4, :])
    v_sb = [pool.tile([128, E], DT, tag=f"v{jt}", name=f"v{jt}") for jt in range(NT)]

    ebias = pool.tile([128, 1], F32, tag="ebias")
    nc.gpsimd.memset(ebias[:], EXP_BIAS)
    ones = pool.tile([128, 128], DT, tag="ones")
    nc.gpsimd.memset(ones[:], 1.0)
    rot = pool.tile([128, 128], DT, tag="rot")
    nc.sync.dma_start(rot[:], rotT)

    for icj in range(NI):
        vps = [ps.tile([128, 512], F32, tag="mm", name=f"vp{jt}") for jt in range(4)]
        for mt in range(NT):
            if mt % 4 == 0:
                xq = first_xq if (icj == 0 and mt == 0) else load_quad(icj, mt // 4)
            for jt in range(4):
                nc.tensor.matmul(
                    vps[jt][:],
                    xq[:, mt % 4, jt * 128 : (jt + 1) * 128],
                    wv_sb[:, mt, :],
                    start=(mt == 0),
                    stop=(mt == NT - 1),
                )
        for jt in range(4):
            nc.scalar.copy(v_sb[4 * icj + jt][:], vps[jt][:])
        if icj == 0:
            # prefetch q/k weights and rope tables behind phase A's compute
            cos_sb = pool.tile([128, L], F32, tag="cos")
            sin_sb = pool.tile([128, L], F32, tag="sin")
            nc.sync.dma_start(cos_sb[:], cosT)
            nc.sync.dma_start(sin_sb[:], sinT)
            wq_sb = pool.tile([128, NT, 512], DT, tag="wq")
            wk_sb = pool.tile([128, NT, 512], DT, tag="wk")
        if icj in (1, 2):
            c0 = 2 * (icj - 1)
            for c in (c0, c0 + 1):
                nc.sync.dma_start(wq_sb[:, 4 * c : 4 * c + 4, :], wqTr[:, 4 * c : 4 * c + 4, :])
                nc.sync.dma_start(wk_sb[:, 4 * c : 4 * c + 4, :], wkTr[:, 4 * c : 4 * c + 4, :])

    # ---- phases B/C/D interleaved at the 512-column block level ---------
    qr = [pool.tile([128, L], DT, tag=f"qr{h}", name=f"qr{h}") for h in range(GH)]
    kr = [pool.tile([128, L], DT, tag=f"kr{h}", name=f"kr{h}") for h in range(GH)]
    ot = [pool.tile([128, L], DT, tag=f"ot{h}", name=f"ot{h}") for h in range(GH)]
    e_tiles = [pool.tile([128, 512], DT, tag=f"E{jt}", name=f"et{jt}") for jt in range(NT)]
    wo_sb = pool.tile([128, GH, L], DT, tag="wo")
    for s_ in range(GH):
        nc.sync.dma_start(wo_sb[:, s_, :], woTr[:, s_, :])

    def emit_rope(batch):
        for pre, dst, dt, isl_ in batch:
            rp = ps.tile([128, 512], F32, tag="acc", bufs=2)
            nc.tensor.matmul(rp[:], rot[:], pre[:], start=True, stop=True)
            t1 = work.tile([128, 512], F32, tag="t1")
            nc.vector.tensor_tensor(t1[:], pre[:], cos_sb[:, isl_], MULT)
            t2 = work.tile([128, 512], F32, tag="t2")
            nc.vector.tensor_tensor(t2[:], rp[:], sin_sb[:, isl_], MULT)
            nc.vector.tensor_tensor(dst[dt][:, isl_], t1[:], t2[:], ADD)

    for ic in range(NI):
        # -- B: q then k projection for this column block --
        isl = slice(ic * 512, (ic + 1) * 512)
        batches = []
        for w_sb, dst in ((wq_sb, qr), (wk_sb, kr)):
            pps = [ps.tile([128, 512], F32, tag="mm", name=f"pp{dt}") for dt in range(GH)]
            for mt in range(NT):
                if mt % 4 == 0:
                    xq = load_quad(ic, mt // 4)
                for dt in range(GH):
                    nc.tensor.matmul(
                        pps[dt][:],
                        w_sb[:, mt, dt * 128 : (dt + 1) * 128],
                        xq[:, mt % 4, :],
                        start=(mt == 0),
                        stop=(mt == NT - 1),
                    )
            batch = []
            for dt in range(GH):
                pre = work.tile([128, 512], DT, tag="pre", bufs=4)
                if dt % 2 == 0:
                    nc.scalar.copy(pre[:], pps[dt][:])
                else:
                    nc.vector.tensor_copy(pre[:], pps[dt][:])
                batch.append((pre, dst, dt, isl))
            batches.append(batch)
            if len(batches) == 2:
                emit_rope(batches[0])  # q rope: its pre tiles finished during k group
        emit_rope(batches[1])

        # -- C: attention for query block I = ic, all heads --
        I = ic
        nj = (I + 1) * 4
        i0 = I * 512

        def vc0(jt):
            # diag tile jt = I*4 + t has valid columns [128*t, 512) only
            return max(0, (jt - I * 4) * 128)

        for h in range(GH):
            for jt in range(nj):
                c0_ = vc0(jt)
                st = ps.tile([128, 512], F32, tag="mm", name="st")
                nc.tensor.matmul(
                    st[:, c0_:],
                    kr[h][:, jt * 128 : (jt + 1) * 128],
                    qr[h][:, i0 + c0_ : i0 + 512],
                    start=True,
                    stop=True,
                )
                et = e_tiles[jt]
                nc.scalar.activation(
                    et[:, c0_:], st[:, c0_:], mybir.ActivationFunctionType.Exp,
                    scale=SCALE, bias=ebias[:],
                )
                if jt >= I * 4:
                    # within valid cols keep upper triangle: c' - p >= 0
                    nc.gpsimd.affine_select(
                        out=et[:, c0_:],
                        in_=et[:, c0_:],
                        compare_op=IS_GE,
                        fill=0.0,
                        base=0,
                        pattern=[[1, 512 - c0_]],
                        channel_multiplier=-1,
                    )
            ovtag, dntag = ("dn", "acc") if h % 2 == 0 else ("acc", "dn")
            ov = ps.tile([128, 512], F32, tag=ovtag, bufs=2)
            for jt in range(nj):
                c0_ = vc0(jt)
                nc.tensor.matmul(
                    ov[:, c0_:],
                    v_sb[jt][:, h * 128 : (h + 1) * 128],
                    e_tiles[jt][:, c0_:],
                    start=(jt == 0),
                    stop=(jt == nj - 1),
                )
            dn = ps.tile([128, 512], F32, tag=dntag, bufs=2)
            for jt in range(nj):
                c0_ = vc0(jt)
                nc.tensor.matmul(
                    dn[:, c0_:], ones[:], e_tiles[jt][:, c0_:],
                    start=(jt == 0), stop=(jt == nj - 1)
                )
            rbi = dpool.tile([128, 512], F32, tag="rbi", bufs=2)
            nc.vector.reciprocal_approx_fast(out=rbi[:], in_=dn[:])
            nc.vector.tensor_tensor(ot[h][:, i0 : i0 + 512], ov[:], rbi[:], MULT)

        # -- D: Wo blocks for query tiles completed by this block --
        dtags = [("mm", 4), ("mm", 4), ("acc", 2), ("dn", 2)]
        for it in range(I * 4, I * 4 + 4):
            for fp in range(2):  # fc pairs
                ob = obpool.tile([128, 1024], F32, tag="ob", bufs=2)
                for half in range(2):
                    fc = 2 * fp + half
                    dtag, dbufs = dtags[(it * NI + fc) % 4]
                    op = ps.tile([128, 512], F32, tag=dtag, bufs=dbufs, name="op")
                    for h in range(GH):
                        nc.tensor.matmul(
                            op[:],
                            ot[h][:, it * 128 : (it + 1) * 128],
                            wo_sb[:, h, fc * 512 : (fc + 1) * 512],
                            start=(h == 0),
                            stop=(h == GH - 1),
                        )
                    if (it + fc) % 2 == 0:
                        nc.vector.tensor_copy(ob[:, half * 512 : (half + 1) * 512], op[:])
                    else:
                        nc.scalar.copy(ob[:, half * 512 : (half + 1) * 512], op[:])
                nc.sync.dma_start(
                    out[it * 128 : (it + 1) * 128, fp * 1024 : (fp + 1) * 1024], ob[:]
                )


def build():
    import contextlib

    nc = bacc.Bacc("TRN2", target_bir_lowering=False, debug=False, num_devices=NCORES)
    io = {
        "xT": nc.dram_tensor("xT", [HID, L], DT, kind="ExternalInput").ap(),
        "wqT": nc.dram_tensor("wqT", [HID, E], DT, kind="ExternalInput").ap(),
        "wkT": nc.dram_tensor("wkT", [HID, E], DT, kind="ExternalInput").ap(),
        "wvT": nc.dram_tensor("wvT", [HID, E], DT, kind="ExternalInput").ap(),
        "woT": nc.dram_tensor("woT", [E, HID], DT, kind="ExternalInput").ap(),
        "cosT": nc.dram_tensor("cosT", [D, L], F32, kind="ExternalInput").ap(),
        "sinT": nc.dram_tensor("sinT", [D, L], F32, kind="ExternalInput").ap(),
        "rotT": nc.dram_tensor("rotT", [D, D], DT, kind="ExternalInput").ap(),
        "out": nc.dram_tensor("out", [L, HID], F32, kind="ExternalOutput").ap(),
    }
    with tile.TileContext(nc) as tc:
        with contextlib.ExitStack() as ctx:
            _emit(nc, tc, ctx, io)
    nc.compile()
    return nc


_NC_CACHE = []


def _rot_matrix():
    # lhsT for the rotate_half matmul: rot(q) = P @ q, lhsT = P^T.
    rotT = np.zeros((D, D), dtype=NP_DT)
    for d in range(D // 2):
        rotT[d, d + 64] = 1.0
        rotT[d + 64, d] = -1.0
    return rotT


def make_in_maps(hidden_states, cos, sin, Wq, Wk, Wv, Wo):
    f = NP_DT
    cosT = np.ascontiguousarray(cos.T.astype(np.float32))
    sinT = np.ascontiguousarray(sin.T.astype(np.float32))
    rotT = _rot_matrix()
    xTs = [np.ascontiguousarray(hidden_states[b].T.astype(f)) for b in range(B)]
    in_maps = []
    for c in range(NCORES):
        b, g = divmod(c, 4)
        sl = slice(g * E, (g + 1) * E)
        in_maps.append({
            "xT": xTs[b],
            "wqT": np.ascontiguousarray(Wq[sl].T.astype(f)),
            "wkT": np.ascontiguousarray(Wk[sl].T.astype(f)),
            "wvT": np.ascontiguousarray(Wv[sl].T.astype(f)),
            "woT": np.ascontiguousarray(Wo[:, sl].T.astype(f)),
            "cosT": cosT,
            "sinT": sinT,
            "rotT": rotT,
        })
    return in_maps


def kernel(hidden_states, cos, sin, Wq, Wk, Wv, Wo):
    hidden_states, cos, sin, Wq, Wk, Wv, Wo = (
        np.asarray(a) for a in (hidden_states, cos, sin, Wq, Wk, Wv, Wo)
    )
    if not _NC_CACHE:
        _NC_CACHE.append(build())
    nc = _NC_CACHE[0]
    in_maps = make_in_maps(hidden_states, cos, sin, Wq, Wk, Wv, Wo)
    r = run_bass_kernel_spmd(nc, in_maps, list(range(NCORES)))
    out = np.empty((B, L, HID), np.float32)
    for b in range(B):
        acc = r.results[4 * b]["out"].astype(np.float32).copy()
        for g in range(1, 4):
            acc += r.results[4 * b + g]["out"]
        out[b] = acc
    return out
